# revision 12
# baseline (speedup 1.0000x reference)
"""Trainium2 Bass kernel for nn_EnhancedHeterogeneousGNN.

Strategy: 8 cores, core c computes the full 6-layer stack for batch c%4
(cores 4-7 duplicate; no collectives — collective latency floor would
dominate). Feature-major activations, f32r matmuls. The U*T x U*T
cross-attention collapses to U queries because the reference broadcasts u
over T; tp is built as broadcast(task part) + edge projection. Weights are
packed on host into per-layer chunk blobs (one DMA per layer chunk).

PSUM budget (8 banks): ps_big [128,1024]x1 = 2, ps_tr [128,128]x1 = 1,
ps_c [128,128]x2 = 2, ps_sm [128,64]x2 = 2, ps_s [128,128]x1 = 1.
"""
import sys
sys.path.insert(0, "/opt/trn_rl_repo")
import math
import numpy as np

import concourse.bacc as bacc
import concourse.tile as tile
import concourse.mybir as mybir
from concourse.bass_utils import run_bass_kernel_spmd

# f32r disabled: ~1e-4 rounding flips floor(u*100) PE-gather indices
F32R = mybir.dt.float32
F32 = mybir.dt.float32
I32 = mybir.dt.int32
AF = mybir.ActivationFunctionType
ALU = mybir.AluOpType
AX = mybir.AxisListType

E = 128; H = 4; HD = 32; L = 6
B, U, T = 4, 16, 64
UT = U * T
SCL = 1.0 / math.sqrt(HD)
N_CORES = 8


# ---------------------------------------------------------------- layout ----
class Chunk:
    def __init__(self, name, dt):
        self.name = name
        self.dt = dt
        self.entries = {}
        self.cols = 0

    def add(self, ename, rows, cols):
        self.entries[ename] = (rows, self.cols, cols)
        self.cols += cols

    def ap(self, tiles, ename, r=None, c=None):
        rows, off, cols = self.entries[ename]
        t = tiles[self.name]
        r0, r1 = (0, rows) if r is None else r
        c0, c1 = (0, cols) if c is None else c
        return t[r0:r1, off + c0:off + c1]


def build_chunks():
    cs = {}

    def C(name, dt):
        cs[name] = Chunk(name, dt)
        return cs[name]

    wc = C("wc", F32R)
    wc.add("ident", 128, 128)
    wc.add("ones_col", 128, 1)
    wc.add("ones_row", 1, 128)
    wc.add("petab", 128, 256)   # rows 0:100 used; table chunk c at cols 128c
    ac = C("ac", F32)
    ac.add("iota", 16, 200)

    for i in range(L):
        kin = 4 if i == 0 else 128
        w = C(f"wu{i}", F32R)
        w.add("uproj", kin, 128)
        for p in ("sa", "ca"):
            for m in ("wq", "wk", "wv", "wo"):
                w.add(f"{p}_{m}", 128, 128)
        w.add("wt_t", kin, 128)
        w.add("wt_e", 4, 128)
        w.add("ffn1", 128, 256)
        w.add("ffn2", 128, 256)  # k-chunk kc at cols 128*kc (contract 256)
        a = C(f"au{i}", F32)
        a.add("uproj_b", 128, 1)
        a.add("sa_wo_b", 128, 1)
        a.add("sa_ln", 128, 2)
        a.add("ca_wo_b", 128, 1)
        a.add("ca_ln", 128, 2)
        a.add("tp_b", 128, 1)
        a.add("ffn1_b", 128, 2)
        a.add("ffn2_b", 128, 1)
        a.add("ln", 128, 2)

        w = C(f"wt{i}", F32R)
        w.add("tproj", kin, 128)
        for m in ("wq", "wk", "wv", "wo"):
            w.add(f"sa_{m}", 128, 128)
        w.add("ua_wq", 128, 128)
        w.add("ua_wku", 128, 128)
        w.add("ua_wvu", 128, 128)
        w.add("ua_wo", 128, 128)
        w.add("ffn1", 128, 512)           # out-chunk oc at cols 128*oc
        w.add("ffn2", 128, 1024)          # tile (kc, oc) at cols 128*(kc*2+oc)
        w.add("ffn3", 128, 256)           # k-chunk kc at cols 128*kc
        a = C(f"at{i}", F32)
        a.add("tproj_b", 128, 1)
        a.add("sa_wo_b", 128, 1)
        a.add("sa_ln", 128, 2)
        a.add("ua_wo_b", 128, 1)
        a.add("ua_kb", 128, 1)
        a.add("ua_vb", 128, 1)
        a.add("ua_ln", 128, 2)
        a.add("ffn1_b", 128, 4)
        a.add("ffn2_b", 128, 2)
        a.add("ffn3_b", 128, 1)
        a.add("ln", 128, 2)

    w = C("wp", F32R)
    w.add("a1", 128, 64)
    w.add("a2", 64, 1)
    w.add("pout", 128, 512)   # tile (kc, oc) at cols 128*(kc*2+oc)
    w.add("el1", 128, 512)
    w.add("el2", 128, 512)
    a = C("ap_", F32)
    a.add("a1_b", 64, 1)
    a.add("a2_b", 1, 1)
    a.add("pout_b", 128, 2)
    a.add("pln", 128, 4)      # g cols 0:2, b cols 2:4
    a.add("el1_b", 128, 2)
    a.add("el2_b", 128, 2)
    return cs


CHUNKS = build_chunks()

BATCH_INPUTS = {
    "edgesT": ([4, UT], F32R),
    "u0T": ([4, U], F32R),
    "t0T": ([4, T], F32R),
    "pe_u0": ([128, U], F32),
    "pe_task": ([128, T], F32),
    "knn": ([128, T], F32),
}
OUTPUTS = {
    "u_out": ([U, 128], F32),
    "t_out": ([T, 128], F32),
    "g_out": ([256, 1], F32),
}


# ------------------------------------------------------------- emission ----
def emit(nc, tc, ctx, aps):
    pool = ctx.enter_context(tc.tile_pool(name="sb", bufs=1))
    pool2 = ctx.enter_context(tc.tile_pool(name="sb2", bufs=1))
    p_big = ctx.enter_context(tc.tile_pool(name="ps_big", bufs=1, space="PSUM"))
    p_tr = ctx.enter_context(tc.tile_pool(name="ps_tr", bufs=1, space="PSUM"))
    p_c = ctx.enter_context(tc.tile_pool(name="ps_c", bufs=2, space="PSUM"))
    p_sm = ctx.enter_context(tc.tile_pool(name="ps_sm", bufs=2, space="PSUM"))
    p_s = ctx.enter_context(tc.tile_pool(name="ps_s", bufs=1, space="PSUM"))

    def ps_big():
        return p_big.tile([128, UT], F32, tag="ps_big", name="ps_big")

    def ps_tr():
        return p_tr.tile([128, 128], F32R, tag="ps_tr", name="ps_tr")

    def ps_c():
        return p_c.tile([128, 128], F32, tag="ps_c", name="ps_c")

    def ps_sm():
        return p_sm.tile([128, 64], F32, tag="ps_sm", name="ps_sm")

    def ps_s():
        return p_s.tile([128, 128], F32, tag="ps_s", name="ps_s")

    tiles = {}
    for cname, ch in CHUNKS.items():
        t = pool.tile([128, ch.cols], ch.dt, tag=f"w_{cname}", name=f"w_{cname}")
        nc.sync.dma_start(t[:], aps[cname][:])
        tiles[cname] = t
    binp = {}
    for name, (shape, dt) in BATCH_INPUTS.items():
        t = pool.tile(shape, dt, tag=f"in_{name}", name=f"in_{name}")
        nc.sync.dma_start(t[:], aps[name][:])
        binp[name] = t

    def W(c, e, r=None, co=None):
        return CHUNKS[c].ap(tiles, e, r, co)

    ident = W("wc", "ident")
    ones_col = W("wc", "ones_col")
    ones_row = W("wc", "ones_row")

    def mm(out, lhsT, rhs, start=True, stop=True):
        nc.tensor.matmul(out, lhsT, rhs, start=start, stop=stop)

    def sb(shape, dt=F32R, tag=None, p=None):
        return (p or pool2).tile(shape, dt, tag=tag, name=tag)

    def proj(w_ap, xT, tok, tag, bias=None):
        """f32r [128, tok] = w_ap.T @ xT (+bias)."""
        pp = ps_sm()
        mm(pp[:, 0:tok], w_ap, xT)
        out = sb([128, tok], F32R, tag=tag)
        if bias is None:
            nc.scalar.activation(out[:], pp[:, 0:tok], AF.Copy)
        else:
            nc.scalar.activation(out[:], pp[:, 0:tok], AF.Identity, bias=bias)
        return out

    def transpose(src_ap, pr, fr, tag):
        """src [pr, fr] f32r -> SBUF f32r [fr, pr] via PE transpose."""
        pt = ps_tr()
        nc.tensor.transpose(pt[0:fr, 0:pr], src_ap, ident[0:pr, 0:pr])
        out = sb([fr, pr], F32R, tag=tag)
        nc.scalar.activation(out[:], pt[0:fr, 0:pr].bitcast(F32), AF.Copy)
        return out

    def ln_fm(xT_ap, tok, gb_ap, tag):
        """LayerNorm over 128 features, feature-major. xT f32r [128,tok]."""
        sp = ps_s()
        mm(sp[0:1, 0:tok], ones_col, xT_ap)
        x2 = sb([128, tok], F32R, tag=f"{tag}_x2")
        nc.scalar.activation(x2[:], xT_ap.bitcast(F32), AF.Square)
        mm(sp[0:1, tok:2 * tok], ones_col, x2[:])
        st = sb([1, 4 * tok], F32, tag=f"{tag}_st")
        mean_r = st[0:1, 0:tok]
        msq_r = st[0:1, tok:2 * tok]
        var_r = st[0:1, 2 * tok:3 * tok]
        std_r = st[0:1, 3 * tok:4 * tok]
        nc.scalar.activation(mean_r, sp[0:1, 0:tok], AF.Copy, scale=1.0 / 128)
        nc.scalar.activation(msq_r, sp[0:1, tok:2 * tok], AF.Copy, scale=1.0 / 128)
        nc.vector.tensor_tensor(var_r, mean_r, mean_r, op=ALU.mult)
        nc.vector.tensor_tensor(var_r, msq_r, var_r, op=ALU.subtract)
        nc.scalar.activation(std_r, var_r, AF.Sqrt, bias=1e-5)
        rst = sb([1, 2 * tok], F32R, tag=f"{tag}_rs")
        nc.vector.reciprocal(rst[0:1, 0:tok], std_r)
        nc.vector.tensor_copy(rst[0:1, tok:2 * tok], mean_r)
        mb = ps_c()
        mm(mb[:, 0:tok], ones_row, rst[0:1, tok:2 * tok])   # mean bcast
        mm(mb[:, tok:2 * tok], ones_row, rst[0:1, 0:tok])   # rstd bcast
        xn = sb([128, tok], F32, tag=f"{tag}_xn")
        nc.vector.tensor_tensor(xn[:], xT_ap.bitcast(F32), mb[:, 0:tok],
                                op=ALU.subtract)
        nc.vector.tensor_tensor(xn[:], xn[:], mb[:, tok:2 * tok], op=ALU.mult)
        out = sb([128, tok], F32R, tag=f"{tag}_out")
        nc.vector.tensor_scalar(out[:], xn[:], gb_ap[:, 0:1], gb_ap[:, 1:2],
                                op0=ALU.mult, op1=ALU.add)
        return out

    def softmax_inplace(S_ap, P, N, tag, mask_ap=None, p_=None):
        """S_ap psum/sbuf [P, N] scores -> A f32r [P, N] SBUF, softmaxed."""
        if mask_ap is not None:
            Sm = sb([P, N], F32, tag=f"{tag}_sm")
            nc.vector.tensor_tensor(Sm[:], S_ap, mask_ap, op=ALU.add)
            src = Sm[:]
        else:
            src = S_ap
        nmx = sb([P, 1], F32, tag=f"{tag}_mx")
        nc.vector.tensor_reduce(nmx[:], src, axis=AX.X, op=ALU.max, negate=True)
        A = sb([P, N], F32R, tag=f"{tag}_A", p=p_)
        ssum = sb([P, 1], F32, tag=f"{tag}_su")
        nc.scalar.activation(A[:], src, AF.Exp, bias=nmx[:], accum_out=ssum[:])
        rsum = sb([P, 1], F32, tag=f"{tag}_rs")
        nc.vector.reciprocal(rsum[:], ssum[:])
        nc.vector.tensor_scalar(A[:], A[:].bitcast(F32), rsum[:], None,
                                op0=ALU.mult)
        return A

    def build_qbd(q_ps_ap, nq, tag):
        """Scaled block-diag Q tiles from psum [128,nq]."""
        out = []
        ngrp = 2 if nq == 64 else 1
        hp = 4 // ngrp
        for g in range(ngrp):
            t = sb([128, hp * nq], F32R, tag=f"{tag}{g}")
            nc.vector.memset(t[:].bitcast(F32), 0.0)
            for hh in range(hp):
                h = g * hp + hh
                nc.scalar.activation(t[32 * h:32 * h + 32, hh * nq:(hh + 1) * nq],
                                     q_ps_ap[32 * h:32 * h + 32, 0:nq],
                                     AF.Copy, scale=SCL)
            out.append(t)
        return out

    def attention(xq_T, nq, KT, VT, nk, wq_ap, wo_ap, wo_b, ln_ap, tag,
                  mask_ap=None):
        """MHA, query input xq_T f32r [128,nq]; K/V f32r [128,nk] SBUF tiles.
        Returns LN(wo(attn) + xq_T) f32r [128, nq] tile."""
        qp = ps_sm()
        mm(qp[:, 0:nq], wq_ap, xq_T)
        qbd = build_qbd(qp, nq, f"{tag}_qbd")
        vtm = transpose(VT[:], 128, nk, f"{tag}_vtm")
        att = sb([128, nq], F32R, tag=f"{tag}_att")
        ngrp = len(qbd)
        hp = 4 // ngrp
        for g in range(ngrp):
            sp = ps_c()
            mm(sp[0:hp * nq, 0:nk], qbd[g][:], KT[:])
            A = softmax_inplace(sp[0:hp * nq, 0:nk], hp * nq, nk,
                                f"{tag}_s{g}", mask_ap)
            ATs = transpose(A[:], hp * nq, nk, f"{tag}_ats")
            avp = ps_c()
            mm(avp[0:128, 0:hp * nq], vtm[:], ATs[:])
            for hh in range(hp):
                h = g * hp + hh
                nc.scalar.activation(
                    att[32 * h:32 * h + 32, 0:nq],
                    avp[32 * h:32 * h + 32, hh * nq:(hh + 1) * nq], AF.Copy)
        op_ = ps_sm()
        mm(op_[:, 0:nq], wo_ap, att[:])
        ob = sb([128, nq], F32, tag=f"{tag}_ob")
        nc.scalar.activation(ob[:], op_[:, 0:nq], AF.Identity, bias=wo_b)
        res = sb([128, nq], F32R, tag=f"{tag}_res")
        nc.vector.tensor_tensor(res[:], ob[:], xq_T.bitcast(F32), op=ALU.add)
        return ln_fm(res[:], nq, ln_ap, f"{tag}_ln")

    # ------------------------------------------------ the 6 layer pairs ----
    uT = None
    tT = None

    for i in range(L):
        wu, au, wt, at = f"wu{i}", f"au{i}", f"wt{i}", f"at{i}"
        kin = 4 if i == 0 else 128
        u_in = binp["u0T"][:] if i == 0 else uT[:]
        t_in = binp["t0T"][:] if i == 0 else tT[:]

        # ---------------- usv layer ----------------
        yp = ps_sm()
        mm(yp[:, 0:U], W(wu, "uproj", r=(0, kin)), u_in)
        y1 = sb([128, U], F32, tag="u_y1")
        nc.scalar.activation(y1[:], yp[:, 0:U], AF.Identity,
                             bias=W(au, "uproj_b"))
        u_pe = sb([128, U], F32R, tag="u_pe")
        if i == 0:
            nc.vector.tensor_tensor(u_pe[:], y1[:], binp["pe_u0"][:], op=ALU.add)
        else:
            pos = sb([2, U], F32, tag="pe_pos")
            nc.vector.tensor_scalar(pos[:], uT[0:2, :].bitcast(F32), 100.0,
                                    None, op0=ALU.mult)
            ii = sb([2, U], I32, tag="pe_ii")
            nc.vector.tensor_copy(ii[:], pos[:])
            rr = sb([2, U], F32, tag="pe_rr")
            nc.vector.tensor_copy(rr[:], ii[:])
            gg = sb([2, U], F32, tag="pe_gg")
            nc.vector.tensor_tensor(gg[:], rr[:], pos[:], op=ALU.is_gt)
            nc.vector.tensor_tensor(rr[:], rr[:], gg[:], op=ALU.subtract)
            idx = sb([2, U], F32R, tag="pe_idx")
            nc.vector.tensor_scalar(idx[:], rr[:], 0.0, 199.0,
                                    op0=ALU.max, op1=ALU.min)
            itp = ps_s()
            mm(itp[0:U, 0:2], idx[:], ident[0:2, 0:2])
            idx_tm = sb([U, 2], F32, tag="pe_idxtm")
            nc.scalar.activation(idx_tm[:], itp[0:U, 0:2], AF.Copy)
            oh = sb([16, 200], F32R, tag="pe_oh")
            nc.vector.tensor_scalar(oh[:], W("ac", "iota"), idx_tm[:, 0:1],
                                    None, op0=ALU.is_equal)
            oh2 = sb([16, 200], F32, tag="pe_oh2")
            nc.vector.tensor_scalar(oh2[:], W("ac", "iota"), idx_tm[:, 1:2],
                                    None, op0=ALU.is_equal)
            nc.vector.tensor_tensor(oh[:], oh[:].bitcast(F32), oh2[:],
                                    op=ALU.add)
            ohT = sb([100, 32], F32R, tag="pe_ohT")
            for c in range(2):
                tp_ = ps_s()
                mm(tp_[0:100, 0:16], oh[:, 100 * c:100 * (c + 1)],
                   ident[0:16, 0:16])
                nc.scalar.activation(ohT[0:100, 16 * c:16 * (c + 1)],
                                     tp_[0:100, 0:16], AF.Copy)
            pep = ps_sm()
            for c in range(2):
                mm(pep[:, 0:U],
                   W("wc", "petab", r=(0, 100), co=(128 * c, 128 * (c + 1))),
                   ohT[0:100, 16 * c:16 * (c + 1)],
                   start=(c == 0), stop=(c == 1))
            nc.vector.tensor_tensor(u_pe[:], y1[:], pep[:, 0:U], op=ALU.add)

        u_sa = attention(
            u_pe[:], U,
            KT=proj(W(wu, "sa_wk"), u_pe[:], U, "u_sak"),
            VT=proj(W(wu, "sa_wv"), u_pe[:], U, "u_sav"),
            nk=U, wq_ap=W(wu, "sa_wq"), wo_ap=W(wu, "sa_wo"),
            wo_b=W(au, "sa_wo_b"), ln_ap=W(au, "sa_ln"), tag="u_sa")

        # cross attention over UT kv tokens
        tpb_p = ps_sm()
        mm(tpb_p[:, 0:T], W(wu, "wt_t", r=(0, kin)), t_in)
        tpb = sb([128, T], F32, tag="u_tpb")
        nc.scalar.activation(tpb[:], tpb_p[:, 0:T], AF.Identity,
                             bias=W(au, "tp_b"))
        ep = ps_big()
        for c in range(2):
            mm(ep[:, 512 * c:512 * (c + 1)], W(wu, "wt_e"),
               binp["edgesT"][:, 512 * c:512 * (c + 1)])
        tpT = sb([128, UT], F32R, tag="u_tpT", p=pool)
        nc.vector.tensor_tensor(
            tpT[:].rearrange("p (u t) -> p u t", u=U),
            ep[:].rearrange("p (u t) -> p u t", u=U),
            tpb[:].unsqueeze(1).broadcast_to([128, U, T]), op=ALU.add)
        kp = ps_big()
        for c in range(2):
            mm(kp[:, 512 * c:512 * (c + 1)], W(wu, "ca_wk"),
               tpT[:, 512 * c:512 * (c + 1)])
        KTc = sb([128, UT], F32R, tag="u_KT", p=pool)
        nc.scalar.activation(KTc[:], kp[:], AF.Copy)
        vp = ps_big()
        for c in range(2):
            mm(vp[:, 512 * c:512 * (c + 1)], W(wu, "ca_wv"),
               tpT[:, 512 * c:512 * (c + 1)])
        VTc = sb([128, UT], F32R, tag="u_VT", p=pool)
        nc.scalar.activation(VTc[:], vp[:], AF.Copy)
        vtm = sb([128, UT], F32R, tag="u_vtm", p=pool)
        for c in range(8):
            pt = ps_tr()
            nc.tensor.transpose(pt[:], VTc[:, 128 * c:128 * (c + 1)], ident[:])
            nc.scalar.activation(vtm[:, 128 * c:128 * (c + 1)],
                                 pt[:].bitcast(F32), AF.Copy)
        qp = ps_sm()
        mm(qp[:, 0:U], W(wu, "ca_wq"), u_sa[:])
        qbd = build_qbd(qp, U, "u_ca_qbd")[0]
        scp = ps_big()
        for c in range(2):
            mm(scp[0:64, 512 * c:512 * (c + 1)], qbd[:],
               KTc[:, 512 * c:512 * (c + 1)])
        A = softmax_inplace(scp[0:64, :], 64, UT, "u_ca", p_=pool)
        ATs = sb([128, 512], F32R, tag="u_ATs", p=pool)
        for c in range(8):
            pt = ps_tr()
            nc.tensor.transpose(pt[0:128, 0:64], A[:, 128 * c:128 * (c + 1)],
                                ident[0:64, 0:64])
            nc.scalar.activation(ATs[:, 64 * c:64 * (c + 1)],
                                 pt[0:128, 0:64].bitcast(F32), AF.Copy)
        avp = ps_c()
        for c in range(8):
            mm(avp[0:128, 0:64], vtm[:, 128 * c:128 * (c + 1)],
               ATs[:, 64 * c:64 * (c + 1)], start=(c == 0), stop=(c == 7))
        att = sb([128, U], F32R, tag="u_ca_att")
        for h in range(4):
            nc.scalar.activation(att[32 * h:32 * h + 32, :],
                                 avp[32 * h:32 * h + 32, 16 * h:16 * h + 16],
                                 AF.Copy)
        op_ = ps_sm()
        mm(op_[:, 0:U], W(wu, "ca_wo"), att[:])
        ob = sb([128, U], F32, tag="u_ca_ob")
        nc.scalar.activation(ob[:], op_[:, 0:U], AF.Identity,
                             bias=W(au, "ca_wo_b"))
        res = sb([128, U], F32R, tag="u_ca_res")
        nc.vector.tensor_tensor(res[:], ob[:], u_sa[:].bitcast(F32), op=ALU.add)
        cross = ln_fm(res[:], U, W(au, "ca_ln"), "u_ca_ln")

        # usv ffn: E -> 2E -> E
        h1 = sb([128, 2 * U], F32R, tag="u_h1")
        for oc in range(2):
            hp_ = ps_sm()
            mm(hp_[:, 0:U], W(wu, "ffn1", co=(128 * oc, 128 * (oc + 1))),
               cross[:])
            nc.scalar.activation(h1[:, U * oc:U * (oc + 1)], hp_[:, 0:U],
                                 AF.Gelu, bias=W(au, "ffn1_b", co=(oc, oc + 1)))
        h2p = ps_sm()
        for kc in range(2):
            mm(h2p[:, 0:U], W(wu, "ffn2", co=(128 * kc, 128 * (kc + 1))),
               h1[:, U * kc:U * (kc + 1)], start=(kc == 0), stop=(kc == 1))
        h2 = sb([128, U], F32, tag="u_h2")
        nc.scalar.activation(h2[:], h2p[:, 0:U], AF.Identity,
                             bias=W(au, "ffn2_b"))
        res2 = sb([128, U], F32R, tag="u_fres")
        nc.vector.tensor_tensor(res2[:], h2[:], cross[:].bitcast(F32),
                                op=ALU.add)
        uT = ln_fm(res2[:], U, W(au, "ln"), "u_ln")

        # ---------------- task layer ----------------
        t1p = ps_sm()
        mm(t1p[:, 0:T], W(wt, "tproj", r=(0, kin)), t_in)
        t1 = sb([128, T], F32, tag="t_y1")
        nc.scalar.activation(t1[:], t1p[:, 0:T], AF.Identity,
                             bias=W(at, "tproj_b"))
        t_pe = sb([128, T], F32R, tag="t_pe")
        nc.vector.tensor_tensor(t_pe[:], t1[:], binp["pe_task"][:], op=ALU.add)

        t_sa = attention(
            t_pe[:], T,
            KT=proj(W(wt, "sa_wk"), t_pe[:], T, "t_sak"),
            VT=proj(W(wt, "sa_wv"), t_pe[:], T, "t_sav"),
            nk=T, wq_ap=W(wt, "sa_wq"), wo_ap=W(wt, "sa_wo"),
            wo_b=W(at, "sa_wo_b"), ln_ap=W(at, "sa_ln"), tag="t_sa",
            mask_ap=binp["knn"][:])

        ku = proj(W(wt, "ua_wku"), uT[:], U, "t_uak", bias=W(at, "ua_kb"))
        vu = proj(W(wt, "ua_wvu"), uT[:], U, "t_uav", bias=W(at, "ua_vb"))
        t_ua = attention(
            t_sa[:], T, KT=ku, VT=vu, nk=U,
            wq_ap=W(wt, "ua_wq"), wo_ap=W(wt, "ua_wo"),
            wo_b=W(at, "ua_wo_b"), ln_ap=W(at, "ua_ln"), tag="t_ua")

        # task ffn: E -> 4E -> 2E -> E (gelu between)
        f1 = sb([128, 4 * T], F32R, tag="t_f1", p=pool)
        for oc in range(4):
            fp = ps_sm()
            mm(fp[:, 0:T], W(wt, "ffn1", co=(128 * oc, 128 * (oc + 1))),
               t_ua[:])
            nc.scalar.activation(f1[:, T * oc:T * (oc + 1)], fp[:, 0:T],
                                 AF.Gelu, bias=W(at, "ffn1_b", co=(oc, oc + 1)))
        f2 = sb([128, 2 * T], F32R, tag="t_f2")
        for oc in range(2):
            fp = ps_sm()
            for kc in range(4):
                mm(fp[:, 0:T],
                   W(wt, "ffn2", co=(128 * (kc * 2 + oc), 128 * (kc * 2 + oc + 1))),
                   f1[:, T * kc:T * (kc + 1)], start=(kc == 0), stop=(kc == 3))
            nc.scalar.activation(f2[:, T * oc:T * (oc + 1)], fp[:, 0:T],
                                 AF.Gelu, bias=W(at, "ffn2_b", co=(oc, oc + 1)))
        f3p = ps_sm()
        for kc in range(2):
            mm(f3p[:, 0:T], W(wt, "ffn3", co=(128 * kc, 128 * (kc + 1))),
               f2[:, T * kc:T * (kc + 1)], start=(kc == 0), stop=(kc == 1))
        f3 = sb([128, T], F32, tag="t_f3")
        nc.scalar.activation(f3[:], f3p[:, 0:T], AF.Identity,
                             bias=W(at, "ffn3_b"))
        res3 = sb([128, T], F32R, tag="t_fres")
        nc.vector.tensor_tensor(res3[:], f3[:], t_ua[:].bitcast(F32),
                                op=ALU.add)
        tT = ln_fm(res3[:], T, W(at, "ln"), "t_ln")

    # ------------------------------------------------ pooling + output ----
    def pool_vec(xT, tok, tag, out_ap):
        a1p = ps_sm()
        mm(a1p[0:64, 0:tok], W("wp", "a1"), xT[:])
        a1 = sb([64, tok], F32R, tag=f"{tag}_a1")
        nc.scalar.activation(a1[:], a1p[0:64, 0:tok], AF.Tanh,
                             bias=W("ap_", "a1_b"))
        a2p = ps_s()
        mm(a2p[0:1, 0:tok], W("wp", "a2"), a1[:])
        arow = sb([1, tok], F32, tag=f"{tag}_ar")
        nc.scalar.activation(arow[:], a2p[0:1, 0:tok], AF.Identity,
                             bias=W("ap_", "a2_b"))
        wrow = softmax_inplace(arow[:], 1, tok, f"{tag}_w")
        wbc = ps_c()
        mm(wbc[:, 0:tok], ones_row, wrow[:])
        pm = sb([128, tok], F32, tag=f"{tag}_pm")
        nc.vector.tensor_tensor(pm[:], xT[:].bitcast(F32), wbc[:, 0:tok],
                                op=ALU.mult)
        nc.vector.tensor_reduce(out_ap, pm[:], axis=AX.X, op=ALU.add)

    g2 = sb([128, 2], F32R, tag="g2")
    pool_vec(uT, U, "pl_u", g2[:, 0:1])
    pool_vec(tT, T, "pl_t", g2[:, 1:2])

    def swap2(x2, tag):
        o = sb([128, 2], F32R, tag=tag)
        nc.vector.tensor_copy(o[:, 0:1], x2[:, 1:2].bitcast(F32))
        nc.vector.tensor_copy(o[:, 1:2], x2[:, 0:1].bitcast(F32))
        return o

    def lin256(wname, bname, x2, func, tag):
        """y2 [128,2] = func(W @ x256 + b), x2 cols = chunks."""
        xsw = swap2(x2, f"{tag}_sw")
        y2 = sb([128, 2], F32R, tag=f"{tag}_y")
        for oc in range(2):
            pp = ps_sm()
            mm(pp[:, 0:2], W("wp", wname, co=(128 * oc, 128 * (oc + 1))),
               x2[:], start=True, stop=False)
            mm(pp[:, 0:2], W("wp", wname, co=(128 * (2 + oc), 128 * (3 + oc))),
               xsw[:], start=False, stop=True)
            nc.scalar.activation(y2[:, oc:oc + 1], pp[:, 0:1], func,
                                 bias=W("ap_", bname, co=(oc, oc + 1)))
        return y2

    gy = lin256("pout", "pout_b", g2, AF.Identity, "g_p")
    # LN over 256 on [128,2]
    sp = ps_s()
    mm(sp[0:1, 0:2], ones_col, gy[:])
    sq = sb([128, 2], F32R, tag="g_sq")
    nc.scalar.activation(sq[:], gy[:].bitcast(F32), AF.Square)
    mm(sp[0:1, 2:4], ones_col, sq[:])
    sps = sb([1, 4], F32, tag="g_sp")
    nc.scalar.activation(sps[:], sp[0:1, 0:4], AF.Copy)
    st = sb([1, 6], F32, tag="g_st")
    nc.vector.tensor_tensor(st[0:1, 0:1], sps[0:1, 0:1], sps[0:1, 1:2],
                            op=ALU.add)   # sum
    nc.vector.tensor_tensor(st[0:1, 1:2], sps[0:1, 2:3], sps[0:1, 3:4],
                            op=ALU.add)   # sumsq
    nc.vector.tensor_scalar(st[0:1, 0:2], st[0:1, 0:2], 1.0 / 256, None,
                            op0=ALU.mult)  # mean, meansq
    nc.vector.tensor_tensor(st[0:1, 2:3], st[0:1, 0:1], st[0:1, 0:1],
                            op=ALU.mult)
    nc.vector.tensor_tensor(st[0:1, 2:3], st[0:1, 1:2], st[0:1, 2:3],
                            op=ALU.subtract)  # var
    nc.scalar.activation(st[0:1, 3:4], st[0:1, 2:3], AF.Sqrt, bias=1e-5)
    rs = sb([1, 2], F32R, tag="g_rs")
    nc.vector.tensor_copy(rs[0:1, 0:1], st[0:1, 0:1])     # mean
    nc.vector.reciprocal(rs[0:1, 1:2], st[0:1, 3:4])      # rstd
    mb = ps_c()
    mm(mb[:, 0:2], ones_row, rs[0:1, 0:2])  # col0 mean bcast, col1 rstd bcast
    xn = sb([128, 2], F32, tag="g_xn")
    nc.vector.tensor_tensor(xn[:], gy[:].bitcast(F32),
                            mb[:, 0:1].broadcast_to([128, 2]), op=ALU.subtract)
    nc.vector.tensor_tensor(xn[:], xn[:],
                            mb[:, 1:2].broadcast_to([128, 2]), op=ALU.mult)
    gn = sb([128, 2], F32R, tag="g_n")
    nc.vector.tensor_tensor(xn[:], xn[:], W("ap_", "pln", co=(0, 2)),
                            op=ALU.mult)
    nc.vector.tensor_tensor(gn[:], xn[:], W("ap_", "pln", co=(2, 4)),
                            op=ALU.add)
    e1 = lin256("el1", "el1_b", gn, AF.Gelu, "g_e1")
    e2 = lin256("el2", "el2_b", e1, AF.Identity, "g_e2")
    gfin = sb([128, 2], F32, tag="g_f")
    nc.vector.tensor_tensor(gfin[:], gn[:].bitcast(F32), e2[:].bitcast(F32),
                            op=ALU.add)

    u_tm = transpose(uT[:], 128, U, "out_u")
    t_tm = transpose(tT[:], 128, T, "out_t")
    nc.sync.dma_start(aps["u_out"][:], u_tm[:].bitcast(F32))
    nc.sync.dma_start(aps["t_out"][:], t_tm[:].bitcast(F32))
    nc.sync.dma_start(aps["g_out"].rearrange("(c p) o -> p (c o)", c=2),
                      gfin[:])


# ------------------------------------------------------------ build once ----
_PROG = None


def _build():
    global _PROG
    if _PROG is not None:
        return _PROG
    from contextlib import ExitStack
    nc = bacc.Bacc("TRN2", target_bir_lowering=False, debug=False,
                   num_devices=N_CORES)
    epst = nc.alloc_sbuf_tensor("const-eps", [128, 1], F32)
    nc.gpsimd.memset(epst.ap(), 1e-5)
    nc.const_aps.aps[(F32, 1e-5)] = epst.ap()
    nc.all_engine_barrier()
    aps = {}
    for cname, ch in CHUNKS.items():
        aps[cname] = nc.dram_tensor(cname, [128, ch.cols], ch.dt,
                                    kind="ExternalInput").ap()
    for name, (shape, dt) in BATCH_INPUTS.items():
        aps[name] = nc.dram_tensor(name, shape, dt, kind="ExternalInput").ap()
    for name, (shape, dt) in OUTPUTS.items():
        aps[name] = nc.dram_tensor(name, shape, dt, kind="ExternalOutput").ap()
    with tile.TileContext(nc) as tc:
        with ExitStack() as ctx:
            with nc.allow_low_precision("f32r rounding is intentional"):
                emit(nc, tc, ctx, aps)
    nc.compile()
    _PROG = nc
    return nc


# ------------------------------------------------------------ host prep ----
def _np(x):
    return np.asarray(x, dtype=np.float32)


def _pe_table_np():
    pos = np.arange(200, dtype=np.float32)[:, None]
    div = np.exp(np.arange(0, E, 2, dtype=np.float32) * (-math.log(10000.0) / E))
    pe = np.zeros((200, E), dtype=np.float32)
    pe[:, 0::2] = np.sin(pos * div)
    pe[:, 1::2] = np.cos(pos * div)
    return pe


def pack_inputs(usv_features, task_features, usv_task_edges, params):
    pet = _pe_table_np()
    shared = {}
    for cname, ch in CHUNKS.items():
        shared[cname] = np.zeros((128, ch.cols), np.float32)

    def put(c, e, arr):
        rows, off, cols = CHUNKS[c].entries[e]
        a = _np(arr)
        if a.ndim == 1:
            a = a[:, None]
        assert a.shape == (rows, cols), (c, e, a.shape, (rows, cols))
        shared[c][0:rows, off:off + cols] = a

    put("wc", "ident", np.eye(128, dtype=np.float32))
    put("wc", "ones_col", np.ones((128, 1), np.float32))
    _, off, _ = CHUNKS["wc"].entries["ones_row"]
    shared["wc"][0:1, off:off + 128] = 1.0
    ptab = np.zeros((128, 256), np.float32)
    ptab[0:100, 0:128] = pet[0:100]
    ptab[0:100, 128:256] = pet[100:200]
    put("wc", "petab", ptab)
    _, off, _ = CHUNKS["ac"].entries["iota"]
    shared["ac"][0:16, off:off + 200] = np.arange(200, dtype=np.float32)[None, :]

    for i in range(L):
        pu_, pt_ = params["usv"][i], params["task"][i]
        kin = 4 if i == 0 else 128
        w = _np(pu_["usv_proj"]["w"])
        wpad = np.zeros((kin, 128), np.float32)
        wpad[0:w.shape[1], :] = w.T
        put(f"wu{i}", "uproj", wpad)
        put(f"au{i}", "uproj_b", _np(pu_["usv_proj"]["b"]))
        for p, key in (("sa", "self_attn"), ("ca", "task_attn")):
            mha = pu_[key]
            for m in ("wq", "wk", "wv"):
                put(f"wu{i}", f"{p}_{m}", _np(mha[m]["w"]).T)
            put(f"wu{i}", f"{p}_wo", _np(mha["wo"]["w"]).T)
            put(f"au{i}", f"{p}_wo_b", _np(mha["wo"]["b"]))
            put(f"au{i}", f"{p}_ln",
                np.stack([_np(mha["ln_g"]), _np(mha["ln_b"])], 1))
        wtp = _np(pu_["task_proj"]["w"])
        tin_real = wtp.shape[1] - 3
        wt_t = np.zeros((kin, 128), np.float32)
        wt_t[0:tin_real, :] = wtp[:, 0:tin_real].T
        put(f"wu{i}", "wt_t", wt_t)
        wt_e = np.zeros((4, 128), np.float32)
        wt_e[0:3, :] = wtp[:, tin_real:].T
        put(f"wu{i}", "wt_e", wt_e)
        put(f"au{i}", "tp_b", _np(pu_["task_proj"]["b"]))
        put(f"wu{i}", "ffn1", _np(pu_["ffn1"]["w"]).T)
        put(f"au{i}", "ffn1_b", _np(pu_["ffn1"]["b"]).reshape(2, 128).T)
        put(f"wu{i}", "ffn2", _np(pu_["ffn2"]["w"]).T.reshape(2, 128, 128)
            .transpose(1, 0, 2).reshape(128, 256))
        put(f"au{i}", "ffn2_b", _np(pu_["ffn2"]["b"]))
        put(f"au{i}", "ln", np.stack([_np(pu_["ln_g"]), _np(pu_["ln_b"])], 1))

        w = _np(pt_["task_proj"]["w"])
        wpad = np.zeros((kin, 128), np.float32)
        wpad[0:w.shape[1], :] = w.T
        put(f"wt{i}", "tproj", wpad)
        put(f"at{i}", "tproj_b", _np(pt_["task_proj"]["b"]))
        mha = pt_["self_attn"]
        for m in ("wq", "wk", "wv"):
            put(f"wt{i}", f"sa_{m}", _np(mha[m]["w"]).T)
        put(f"wt{i}", "sa_wo", _np(mha["wo"]["w"]).T)
        put(f"at{i}", "sa_wo_b", _np(mha["wo"]["b"]))
        put(f"at{i}", "sa_ln",
            np.stack([_np(mha["ln_g"]), _np(mha["ln_b"])], 1))
        mha = pt_["usv_attn"]
        wu_ = _np(pt_["usv_proj"]["w"])
        bu_ = _np(pt_["usv_proj"]["b"])
        put(f"wt{i}", "ua_wq", _np(mha["wq"]["w"]).T)
        wk_, wv_ = _np(mha["wk"]["w"]), _np(mha["wv"]["w"])
        put(f"wt{i}", "ua_wku", (wk_ @ wu_).T)
        put(f"at{i}", "ua_kb", wk_ @ bu_)
        put(f"wt{i}", "ua_wvu", (wv_ @ wu_).T)
        put(f"at{i}", "ua_vb", wv_ @ bu_)
        put(f"wt{i}", "ua_wo", _np(mha["wo"]["w"]).T)
        put(f"at{i}", "ua_wo_b", _np(mha["wo"]["b"]))
        put(f"at{i}", "ua_ln",
            np.stack([_np(mha["ln_g"]), _np(mha["ln_b"])], 1))
        put(f"wt{i}", "ffn1", _np(pt_["ffn1"]["w"]).T)
        put(f"at{i}", "ffn1_b", _np(pt_["ffn1"]["b"]).reshape(4, 128).T)
        w2 = _np(pt_["ffn2"]["w"]).T
        put(f"wt{i}", "ffn2", w2.reshape(4, 128, 2, 128).transpose(1, 0, 2, 3)
            .reshape(128, 1024))
        put(f"at{i}", "ffn2_b", _np(pt_["ffn2"]["b"]).reshape(2, 128).T)
        put(f"wt{i}", "ffn3", _np(pt_["ffn3"]["w"]).T.reshape(2, 128, 128)
            .transpose(1, 0, 2).reshape(128, 256))
        put(f"at{i}", "ffn3_b", _np(pt_["ffn3"]["b"]))
        put(f"at{i}", "ln", np.stack([_np(pt_["ln_g"]), _np(pt_["ln_b"])], 1))

    pp = params["pool"]
    put("wp", "a1", _np(pp["a1"]["w"]).T)
    put("ap_", "a1_b", _np(pp["a1"]["b"]))
    put("wp", "a2", _np(pp["a2"]["w"]).T)
    put("ap_", "a2_b", _np(pp["a2"]["b"]).reshape(1, 1))
    put("wp", "pout", _np(pp["out"]["w"]).T.reshape(2, 128, 2, 128)
        .transpose(1, 0, 2, 3).reshape(128, 512))
    put("ap_", "pout_b", _np(pp["out"]["b"]).reshape(2, 128).T)
    put("ap_", "pln", np.concatenate(
        [_np(pp["ln_g"]).reshape(2, 128).T,
         _np(pp["ln_b"]).reshape(2, 128).T], 1))
    for nm, key in (("el1", "l1"), ("el2", "l2")):
        put("wp", nm, _np(params["enh"][key]["w"]).T.reshape(2, 128, 2, 128)
            .transpose(1, 0, 2, 3).reshape(128, 512))
        put("ap_", f"{nm}_b", _np(params["enh"][key]["b"]).reshape(2, 128).T)

    uf = _np(usv_features)
    tf = _np(task_features)
    ed = _np(usv_task_edges)
    per_batch = []
    for b in range(B):
        d = {}
        e = np.zeros((4, UT), np.float32)
        e[0:3, :] = ed[b].reshape(UT, 3).T
        d["edgesT"] = e
        x = np.zeros((4, U), np.float32)
        x[0:3, :] = uf[b].T
        d["u0T"] = x
        x = np.zeros((4, T), np.float32)
        x[0:4, :] = tf[b].T
        d["t0T"] = x
        idx = np.clip((uf[b, :, 0:2] * 100.0).astype(np.int32), 0, 199)
        d["pe_u0"] = (pet[idx[:, 0]] + pet[idx[:, 1]]).T.copy()
        tpos = tf[b, :, 0:2]
        idx = np.clip((tpos * 100.0).astype(np.int32), 0, 199)
        d["pe_task"] = (pet[idx[:, 0]] + pet[idx[:, 1]]).T.copy()
        d2 = ((tpos[:, None, :] - tpos[None, :, :]) ** 2).sum(-1)
        dist = np.sqrt(d2, dtype=np.float32)
        nidx = np.argsort(dist, axis=-1, kind="stable")[:, :5]
        bias = np.full((T, T), -1e30, np.float32)
        np.put_along_axis(bias, nidx, 0.0, axis=-1)
        d["knn"] = np.tile(bias, (2, 1))
        per_batch.append(d)
    return shared, per_batch


# --------------------------------------------------------------- kernel ----
def kernel(usv_features, task_features, usv_task_edges, params):
    nc = _build()
    shared, per_batch = pack_inputs(usv_features, task_features,
                                    usv_task_edges, params)
    in_maps = [{**shared, **per_batch[c % B]} for c in range(N_CORES)]
    res = run_bass_kernel_spmd(nc, in_maps, core_ids=list(range(N_CORES)))
    u = np.stack([res.results[b]["u_out"] for b in range(B)])
    t = np.stack([res.results[b]["t_out"] for b in range(B)])
    g = np.stack([res.results[b]["g_out"][:, 0] for b in range(B)])
    return u, t, g


# revision 15
# speedup vs baseline: 1.0961x; 1.0961x over previous
"""Trainium2 Bass kernel for nn_EnhancedHeterogeneousGNN.

Strategy: 8 cores, core c computes the full 6-layer stack for batch c%4
(cores 4-7 duplicate; no collectives — collective latency floor would
dominate). Feature-major activations, f32r matmuls. The U*T x U*T
cross-attention collapses to U queries because the reference broadcasts u
over T; tp is built as broadcast(task part) + edge projection. Weights are
packed on host into per-layer chunk blobs (one DMA per layer chunk).

PSUM budget (8 banks): ps_big [128,1024]x1 = 2, ps_tr [128,128]x1 = 1,
ps_c [128,128]x2 = 2, ps_sm [128,64]x2 = 2, ps_s [128,128]x1 = 1.
"""
import sys
sys.path.insert(0, "/opt/trn_rl_repo")
import math
import numpy as np

import concourse.bacc as bacc
import concourse.tile as tile
import concourse.mybir as mybir
from concourse.bass_utils import run_bass_kernel_spmd

# f32r disabled: ~1e-4 rounding flips floor(u*100) PE-gather indices
F32R = mybir.dt.float32
F32 = mybir.dt.float32
I32 = mybir.dt.int32
AF = mybir.ActivationFunctionType
ALU = mybir.AluOpType
AX = mybir.AxisListType

E = 128; H = 4; HD = 32; L = 6
B, U, T = 4, 16, 64
UT = U * T
SCL = 1.0 / math.sqrt(HD)
N_CORES = 8


# ---------------------------------------------------------------- layout ----
class Chunk:
    def __init__(self, name, dt):
        self.name = name
        self.dt = dt
        self.entries = {}
        self.cols = 0

    def add(self, ename, rows, cols):
        self.entries[ename] = (rows, self.cols, cols)
        self.cols += cols

    def ap(self, tiles, ename, r=None, c=None):
        rows, off, cols = self.entries[ename]
        t = tiles[self.name]
        r0, r1 = (0, rows) if r is None else r
        c0, c1 = (0, cols) if c is None else c
        return t[r0:r1, off + c0:off + c1]


def build_chunks():
    cs = {}

    def C(name, dt):
        cs[name] = Chunk(name, dt)
        return cs[name]

    wc = C("wc", F32R)
    wc.add("ident", 128, 128)
    wc.add("ones_col", 128, 1)
    wc.add("ones_row", 1, 128)
    wc.add("petab", 128, 256)   # rows 0:100 used; table chunk c at cols 128c
    ac = C("ac", F32)
    ac.add("iota", 16, 200)

    for i in range(L):
        kin = 4 if i == 0 else 128
        w = C(f"wu{i}", F32R)
        w.add("uproj", kin, 128)
        for p in ("sa", "ca"):
            for m in ("wq", "wk", "wv", "wo"):
                w.add(f"{p}_{m}", 128, 128)
        w.add("wt_t", kin, 128)
        w.add("wt_e", 4, 128)
        w.add("ffn1", 128, 256)
        w.add("ffn2", 128, 256)  # k-chunk kc at cols 128*kc (contract 256)
        a = C(f"au{i}", F32)
        a.add("uproj_b", 128, 1)
        a.add("sa_wo_b", 128, 1)
        a.add("sa_ln", 128, 2)
        a.add("ca_wo_b", 128, 1)
        a.add("ca_ln", 128, 2)
        a.add("tp_b", 128, 1)
        a.add("ffn1_b", 128, 2)
        a.add("ffn2_b", 128, 1)
        a.add("ln", 128, 2)

        w = C(f"wt{i}", F32R)
        w.add("tproj", kin, 128)
        for m in ("wq", "wk", "wv", "wo"):
            w.add(f"sa_{m}", 128, 128)
        w.add("ua_wq", 128, 128)
        w.add("ua_wku", 128, 128)
        w.add("ua_wvu", 128, 128)
        w.add("ua_wo", 128, 128)
        w.add("ffn1", 128, 512)           # out-chunk oc at cols 128*oc
        w.add("ffn2", 128, 1024)          # tile (kc, oc) at cols 128*(kc*2+oc)
        w.add("ffn3", 128, 256)           # k-chunk kc at cols 128*kc
        a = C(f"at{i}", F32)
        a.add("tproj_b", 128, 1)
        a.add("sa_wo_b", 128, 1)
        a.add("sa_ln", 128, 2)
        a.add("ua_wo_b", 128, 1)
        a.add("ua_kb", 128, 1)
        a.add("ua_vb", 128, 1)
        a.add("ua_ln", 128, 2)
        a.add("ffn1_b", 128, 4)
        a.add("ffn2_b", 128, 2)
        a.add("ffn3_b", 128, 1)
        a.add("ln", 128, 2)

    w = C("wp", F32R)
    w.add("a1", 128, 64)
    w.add("a2", 64, 1)
    w.add("pout", 128, 512)   # tile (kc, oc) at cols 128*(kc*2+oc)
    w.add("el1", 128, 512)
    w.add("el2", 128, 512)
    a = C("ap_", F32)
    a.add("a1_b", 64, 1)
    a.add("a2_b", 1, 1)
    a.add("pout_b", 128, 2)
    a.add("pln", 128, 4)      # g cols 0:2, b cols 2:4
    a.add("el1_b", 128, 2)
    a.add("el2_b", 128, 2)
    return cs


CHUNKS = build_chunks()

BATCH_INPUTS = {
    "edgesT": ([4, UT], F32R),
    "u0T": ([4, U], F32R),
    "t0T": ([4, T], F32R),
    "pe_u0": ([128, U], F32),
    "pe_task": ([128, T], F32),
    "knn": ([128, T], F32),
}
OUTPUTS = {
    "u_out": ([U, 128], F32),
    "t_out": ([T, 128], F32),
    "g_out": ([256, 1], F32),
}


# ------------------------------------------------------------- emission ----
def emit(nc, tc, ctx, aps):
    pool = ctx.enter_context(tc.tile_pool(name="sb", bufs=1))
    pool2 = ctx.enter_context(tc.tile_pool(name="sb2", bufs=1))
    p_big = ctx.enter_context(tc.tile_pool(name="ps_big", bufs=1, space="PSUM"))
    p_tr = ctx.enter_context(tc.tile_pool(name="ps_tr", bufs=1, space="PSUM"))
    p_c = ctx.enter_context(tc.tile_pool(name="ps_c", bufs=2, space="PSUM"))
    p_sm = ctx.enter_context(tc.tile_pool(name="ps_sm", bufs=2, space="PSUM"))
    p_s = ctx.enter_context(tc.tile_pool(name="ps_s", bufs=1, space="PSUM"))

    def ps_big():
        return p_big.tile([128, UT], F32, tag="ps_big", name="ps_big")

    def ps_tr():
        return p_tr.tile([128, 128], F32R, tag="ps_tr", name="ps_tr")

    def ps_c():
        return p_c.tile([128, 128], F32, tag="ps_c", name="ps_c")

    def ps_sm():
        return p_sm.tile([128, 64], F32, tag="ps_sm", name="ps_sm")

    def ps_s():
        return p_s.tile([128, 128], F32, tag="ps_s", name="ps_s")

    tiles = {}
    for cname, ch in CHUNKS.items():
        t = pool.tile([128, ch.cols], ch.dt, tag=f"w_{cname}", name=f"w_{cname}")
        nc.sync.dma_start(t[:], aps[cname][:])
        tiles[cname] = t
    binp = {}
    for name, (shape, dt) in BATCH_INPUTS.items():
        t = pool.tile(shape, dt, tag=f"in_{name}", name=f"in_{name}")
        nc.sync.dma_start(t[:], aps[name][:])
        binp[name] = t

    def W(c, e, r=None, co=None):
        return CHUNKS[c].ap(tiles, e, r, co)

    ident = W("wc", "ident")
    ones_col = W("wc", "ones_col")
    ones_row = W("wc", "ones_row")

    def mm(out, lhsT, rhs, start=True, stop=True):
        nc.tensor.matmul(out, lhsT, rhs, start=start, stop=stop)

    def sb(shape, dt=F32R, tag=None, p=None):
        return (p or pool2).tile(shape, dt, tag=tag, name=tag)

    def proj(w_ap, xT, tok, tag, bias=None):
        """f32r [128, tok] = w_ap.T @ xT (+bias), copy on DVE."""
        pp = ps_sm()
        mm(pp[:, 0:tok], w_ap, xT)
        out = sb([128, tok], F32R, tag=tag)
        if bias is None:
            nc.vector.tensor_copy(out[:], pp[:, 0:tok])
        else:
            nc.vector.tensor_scalar(out[:], pp[:, 0:tok], bias, None,
                                    op0=ALU.add)
        return out

    def transpose(src_ap, pr, fr, tag):
        """src [pr, fr] f32r -> SBUF f32r [fr, pr] via PE transpose."""
        pt = ps_tr()
        nc.tensor.transpose(pt[0:fr, 0:pr], src_ap, ident[0:pr, 0:pr])
        out = sb([fr, pr], F32R, tag=tag)
        nc.vector.tensor_copy(out[:], pt[0:fr, 0:pr].bitcast(F32))
        return out

    def ln_fm(xT_ap, tok, gb_ap, tag):
        """LayerNorm over 128 features, feature-major. xT f32r [128,tok]."""
        x2 = sb([128, tok], F32R, tag="ln_x2")
        nc.vector.tensor_tensor(x2[:], xT_ap, xT_ap, op=ALU.mult)
        sp = ps_s()
        mm(sp[0:1, 0:tok], ones_col, xT_ap)
        mm(sp[0:1, tok:2 * tok], ones_col, x2[:])
        st = sb([1, 2 * tok], F32R, tag="ln_st")
        mean_r = st[0:1, 0:tok]
        rstd_r = st[0:1, tok:2 * tok]
        nc.vector.tensor_scalar(mean_r, sp[0:1, 0:tok], 1.0 / 128, None,
                                op0=ALU.mult)
        sc = sb([1, 2 * tok], F32, tag="ln_sc")
        msq_r = sc[0:1, 0:tok]
        var_r = sc[0:1, tok:2 * tok]
        nc.vector.tensor_scalar(msq_r, sp[0:1, tok:2 * tok], 1.0 / 128, None,
                                op0=ALU.mult)
        nc.vector.tensor_tensor(var_r, mean_r, mean_r, op=ALU.mult)
        nc.vector.tensor_tensor(var_r, msq_r, var_r, op=ALU.subtract)
        nc.scalar.activation(var_r, var_r, AF.Sqrt, bias=1e-5)
        nc.vector.reciprocal(rstd_r, var_r)
        mb = ps_c()
        mm(mb[:, 0:2 * tok], ones_row, st[0:1, 0:2 * tok])
        xn = sb([128, tok], F32, tag="ln_xn")
        nc.vector.tensor_tensor(xn[:], xT_ap.bitcast(F32), mb[:, 0:tok],
                                op=ALU.subtract)
        nc.vector.tensor_tensor(xn[:], xn[:], mb[:, tok:2 * tok], op=ALU.mult)
        out = sb([128, tok], F32R, tag=f"{tag}_out")
        nc.vector.tensor_scalar(out[:], xn[:], gb_ap[:, 0:1], gb_ap[:, 1:2],
                                op0=ALU.mult, op1=ALU.add)
        return out

    def softmax_inplace(S_ap, P, N, tag, mask_ap=None, p_=None, scale=1.0):
        """scores -> A f32r [P, N] SBUF softmaxed (no max subtraction)."""
        if mask_ap is not None:
            Sm = sb([P, N], F32, tag=f"{tag}_sm")
            nc.vector.tensor_tensor(Sm[:], S_ap, mask_ap, op=ALU.add)
            src = Sm[:]
        else:
            src = S_ap
        A = sb([P, N], F32R, tag=f"{tag}_A", p=p_)
        ssum = sb([P, 1], F32, tag=f"{tag}_su")
        nc.scalar.activation(A[:], src, AF.Exp, bias=0.0, scale=scale,
                             accum_out=ssum[:])
        rsum = sb([P, 1], F32, tag=f"{tag}_rs")
        nc.vector.reciprocal(rsum[:], ssum[:])
        nc.vector.tensor_scalar(A[:], A[:].bitcast(F32), rsum[:], None,
                                op0=ALU.mult)
        return A

    qbd_tiles = {}

    def build_qbd(q_ps_ap, nq, tag):
        """Block-diag Q tiles from psum [128,nq] (scale folded into exp).
        Persistent tiles: off-block zeros written once, reused across layers."""
        out = []
        ngrp = 2 if nq == 64 else 1
        hp = 4 // ngrp
        for g in range(ngrp):
            key = f"{tag}{g}"
            t = qbd_tiles.get(key)
            if t is None:
                t = pool.tile([128, hp * nq], F32R, tag=key, name=key)
                qbd_tiles[key] = t
                nc.vector.memset(t[:].bitcast(F32), 0.0)
            for hh in range(hp):
                h = g * hp + hh
                nc.vector.tensor_copy(
                    t[32 * h:32 * h + 32, hh * nq:(hh + 1) * nq],
                    q_ps_ap[32 * h:32 * h + 32, 0:nq])
            out.append(t)
        return out

    def attention(xq_T, nq, KT, VT, nk, wq_ap, wo_ap, wo_b, ln_ap, tag,
                  mask_ap=None):
        """MHA, query input xq_T f32r [128,nq]; K/V f32r [128,nk] SBUF tiles.
        Returns LN(wo(attn) + xq_T) f32r [128, nq] tile."""
        qp = ps_sm()
        mm(qp[:, 0:nq], wq_ap, xq_T)
        qbd = build_qbd(qp, nq, f"{tag}_qbd")
        vtm = transpose(VT[:], 128, nk, f"{tag}_vtm")
        att = sb([128, nq], F32R, tag=f"{tag}_att")
        ngrp = len(qbd)
        hp = 4 // ngrp
        for g in range(ngrp):
            sp = ps_c()
            mm(sp[0:hp * nq, 0:nk], qbd[g][:], KT[:])
            A = softmax_inplace(sp[0:hp * nq, 0:nk], hp * nq, nk,
                                f"{tag}_s{g}", mask_ap, scale=SCL)
            ATs = transpose(A[:], hp * nq, nk, f"{tag}_ats")
            avp = ps_c()
            mm(avp[0:128, 0:hp * nq], vtm[:], ATs[:])
            for hh in range(hp):
                h = g * hp + hh
                nc.vector.tensor_copy(
                    att[32 * h:32 * h + 32, 0:nq],
                    avp[32 * h:32 * h + 32, hh * nq:(hh + 1) * nq])
        op_ = ps_sm()
        mm(op_[:, 0:nq], wo_ap, att[:])
        ob = sb([128, nq], F32, tag=f"{tag}_ob")
        nc.vector.tensor_scalar(ob[:], op_[:, 0:nq], wo_b, None, op0=ALU.add)
        res = sb([128, nq], F32R, tag=f"{tag}_res")
        nc.vector.tensor_tensor(res[:], ob[:], xq_T.bitcast(F32), op=ALU.add)
        return ln_fm(res[:], nq, ln_ap, f"{tag}_ln")

    # ------------------------------------------------ the 6 layer pairs ----
    uT = None
    tT = None

    for i in range(L):
        wu, au, wt, at = f"wu{i}", f"au{i}", f"wt{i}", f"at{i}"
        kin = 4 if i == 0 else 128
        u_in = binp["u0T"][:] if i == 0 else uT[:]
        t_in = binp["t0T"][:] if i == 0 else tT[:]

        # ---------------- usv layer ----------------
        yp = ps_sm()
        mm(yp[:, 0:U], W(wu, "uproj", r=(0, kin)), u_in)
        y1 = sb([128, U], F32, tag="u_y1")
        nc.vector.tensor_scalar(y1[:], yp[:, 0:U], W(au, "uproj_b"), None,
                                op0=ALU.add)
        u_pe = sb([128, U], F32R, tag="u_pe")
        if i == 0:
            nc.vector.tensor_tensor(u_pe[:], y1[:], binp["pe_u0"][:], op=ALU.add)
        else:
            pos = sb([2, U], F32, tag="pe_pos")
            nc.vector.tensor_scalar(pos[:], uT[0:2, :].bitcast(F32), 100.0,
                                    None, op0=ALU.mult)
            ii = sb([2, U], I32, tag="pe_ii")
            nc.vector.tensor_copy(ii[:], pos[:])
            rr = sb([2, U], F32, tag="pe_rr")
            nc.vector.tensor_copy(rr[:], ii[:])
            gg = sb([2, U], F32, tag="pe_gg")
            nc.vector.tensor_tensor(gg[:], rr[:], pos[:], op=ALU.is_gt)
            nc.vector.tensor_tensor(rr[:], rr[:], gg[:], op=ALU.subtract)
            idx = sb([2, U], F32R, tag="pe_idx")
            nc.vector.tensor_scalar(idx[:], rr[:], 0.0, 199.0,
                                    op0=ALU.max, op1=ALU.min)
            itp = ps_s()
            mm(itp[0:U, 0:2], idx[:], ident[0:2, 0:2])
            idx_tm = sb([U, 2], F32, tag="pe_idxtm")
            nc.vector.tensor_copy(idx_tm[:], itp[0:U, 0:2])
            oh = sb([16, 200], F32R, tag="pe_oh")
            nc.vector.tensor_scalar(oh[:], W("ac", "iota"), idx_tm[:, 0:1],
                                    None, op0=ALU.is_equal)
            oh2 = sb([16, 200], F32, tag="pe_oh2")
            nc.vector.tensor_scalar(oh2[:], W("ac", "iota"), idx_tm[:, 1:2],
                                    None, op0=ALU.is_equal)
            nc.vector.tensor_tensor(oh[:], oh[:].bitcast(F32), oh2[:],
                                    op=ALU.add)
            ohT = sb([100, 32], F32R, tag="pe_ohT")
            for c in range(2):
                tp_ = ps_s()
                mm(tp_[0:100, 0:16], oh[:, 100 * c:100 * (c + 1)],
                   ident[0:16, 0:16])
                nc.vector.tensor_copy(ohT[0:100, 16 * c:16 * (c + 1)],
                                       tp_[0:100, 0:16])
            pep = ps_sm()
            for c in range(2):
                mm(pep[:, 0:U],
                   W("wc", "petab", r=(0, 100), co=(128 * c, 128 * (c + 1))),
                   ohT[0:100, 16 * c:16 * (c + 1)],
                   start=(c == 0), stop=(c == 1))
            nc.vector.tensor_tensor(u_pe[:], y1[:], pep[:, 0:U], op=ALU.add)

        u_sa = attention(
            u_pe[:], U,
            KT=proj(W(wu, "sa_wk"), u_pe[:], U, "u_sak"),
            VT=proj(W(wu, "sa_wv"), u_pe[:], U, "u_sav"),
            nk=U, wq_ap=W(wu, "sa_wq"), wo_ap=W(wu, "sa_wo"),
            wo_b=W(au, "sa_wo_b"), ln_ap=W(au, "sa_ln"), tag="u_sa")

        # cross attention over UT kv tokens
        tpb_p = ps_sm()
        mm(tpb_p[:, 0:T], W(wu, "wt_t", r=(0, kin)), t_in)
        tpb = sb([128, T], F32, tag="u_tpb")
        nc.vector.tensor_scalar(tpb[:], tpb_p[:, 0:T], W(au, "tp_b"), None,
                                op0=ALU.add)
        ep = ps_big()
        for c in range(2):
            mm(ep[:, 512 * c:512 * (c + 1)], W(wu, "wt_e"),
               binp["edgesT"][:, 512 * c:512 * (c + 1)])
        tpT = sb([128, UT], F32R, tag="u_tpT", p=pool)
        nc.vector.tensor_tensor(
            tpT[:].rearrange("p (u t) -> p u t", u=U),
            ep[:].rearrange("p (u t) -> p u t", u=U),
            tpb[:].unsqueeze(1).broadcast_to([128, U, T]), op=ALU.add)
        kp = ps_big()
        for c in range(2):
            mm(kp[:, 512 * c:512 * (c + 1)], W(wu, "ca_wk"),
               tpT[:, 512 * c:512 * (c + 1)])
        KTc = sb([128, UT], F32R, tag="u_KT", p=pool)
        nc.vector.tensor_copy(KTc[:], kp[:])
        # V token-major directly: chunk c = tokens 128c..128c+128
        vtm = sb([128, UT], F32R, tag="u_vtm", p=pool)
        for c in range(8):
            vp = ps_tr()
            mm(vp[:].bitcast(F32), tpT[:, 128 * c:128 * (c + 1)],
               W(wu, "ca_wv"))
            nc.vector.tensor_copy(vtm[:, 128 * c:128 * (c + 1)],
                                  vp[:].bitcast(F32))
        qp = ps_sm()
        mm(qp[:, 0:U], W(wu, "ca_wq"), u_sa[:])
        qbd = build_qbd(qp, U, "u_ca_qbd")[0]
        scp = ps_big()
        for c in range(2):
            mm(scp[0:64, 512 * c:512 * (c + 1)], qbd[:],
               KTc[:, 512 * c:512 * (c + 1)])
        A = softmax_inplace(scp[0:64, :], 64, UT, "u_ca", p_=pool, scale=SCL)
        ATs = sb([128, 512], F32R, tag="u_ATs", p=pool)
        for c in range(8):
            pt = ps_tr()
            nc.tensor.transpose(pt[0:128, 0:64], A[:, 128 * c:128 * (c + 1)],
                                ident[0:64, 0:64])
            nc.vector.tensor_copy(ATs[:, 64 * c:64 * (c + 1)],
                                  pt[0:128, 0:64].bitcast(F32))
        avp = ps_c()
        for c in range(8):
            mm(avp[0:128, 0:64], vtm[:, 128 * c:128 * (c + 1)],
               ATs[:, 64 * c:64 * (c + 1)], start=(c == 0), stop=(c == 7))
        att = sb([128, U], F32R, tag="u_ca_att")
        for h in range(4):
            nc.vector.tensor_copy(att[32 * h:32 * h + 32, :],
                                  avp[32 * h:32 * h + 32, 16 * h:16 * h + 16])
        op_ = ps_sm()
        mm(op_[:, 0:U], W(wu, "ca_wo"), att[:])
        ob = sb([128, U], F32, tag="u_ca_ob")
        nc.vector.tensor_scalar(ob[:], op_[:, 0:U], W(au, "ca_wo_b"), None,
                                op0=ALU.add)
        res = sb([128, U], F32R, tag="u_ca_res")
        nc.vector.tensor_tensor(res[:], ob[:], u_sa[:].bitcast(F32), op=ALU.add)
        cross = ln_fm(res[:], U, W(au, "ca_ln"), "u_ca_ln")

        # usv ffn: E -> 2E -> E
        h1 = sb([128, 2 * U], F32R, tag="u_h1")
        for oc in range(2):
            hp_ = ps_sm()
            mm(hp_[:, 0:U], W(wu, "ffn1", co=(128 * oc, 128 * (oc + 1))),
               cross[:])
            nc.scalar.activation(h1[:, U * oc:U * (oc + 1)], hp_[:, 0:U],
                                 AF.Gelu, bias=W(au, "ffn1_b", co=(oc, oc + 1)))
        h2p = ps_sm()
        for kc in range(2):
            mm(h2p[:, 0:U], W(wu, "ffn2", co=(128 * kc, 128 * (kc + 1))),
               h1[:, U * kc:U * (kc + 1)], start=(kc == 0), stop=(kc == 1))
        h2 = sb([128, U], F32, tag="u_h2")
        nc.vector.tensor_scalar(h2[:], h2p[:, 0:U], W(au, "ffn2_b"), None,
                                op0=ALU.add)
        res2 = sb([128, U], F32R, tag="u_fres")
        nc.vector.tensor_tensor(res2[:], h2[:], cross[:].bitcast(F32),
                                op=ALU.add)
        uT = ln_fm(res2[:], U, W(au, "ln"), "u_ln")

        # ---------------- task layer ----------------
        t1p = ps_sm()
        mm(t1p[:, 0:T], W(wt, "tproj", r=(0, kin)), t_in)
        t1 = sb([128, T], F32, tag="t_y1")
        nc.vector.tensor_scalar(t1[:], t1p[:, 0:T], W(at, "tproj_b"), None,
                                op0=ALU.add)
        t_pe = sb([128, T], F32R, tag="t_pe")
        nc.vector.tensor_tensor(t_pe[:], t1[:], binp["pe_task"][:], op=ALU.add)

        t_sa = attention(
            t_pe[:], T,
            KT=proj(W(wt, "sa_wk"), t_pe[:], T, "t_sak"),
            VT=proj(W(wt, "sa_wv"), t_pe[:], T, "t_sav"),
            nk=T, wq_ap=W(wt, "sa_wq"), wo_ap=W(wt, "sa_wo"),
            wo_b=W(at, "sa_wo_b"), ln_ap=W(at, "sa_ln"), tag="t_sa",
            mask_ap=binp["knn"][:])

        ku = proj(W(wt, "ua_wku"), uT[:], U, "t_uak", bias=W(at, "ua_kb"))
        vu = proj(W(wt, "ua_wvu"), uT[:], U, "t_uav", bias=W(at, "ua_vb"))
        t_ua = attention(
            t_sa[:], T, KT=ku, VT=vu, nk=U,
            wq_ap=W(wt, "ua_wq"), wo_ap=W(wt, "ua_wo"),
            wo_b=W(at, "ua_wo_b"), ln_ap=W(at, "ua_ln"), tag="t_ua")

        # task ffn: E -> 4E -> 2E -> E (gelu between)
        f1 = sb([128, 4 * T], F32R, tag="t_f1", p=pool)
        for oc in range(4):
            fp = ps_sm()
            mm(fp[:, 0:T], W(wt, "ffn1", co=(128 * oc, 128 * (oc + 1))),
               t_ua[:])
            nc.scalar.activation(f1[:, T * oc:T * (oc + 1)], fp[:, 0:T],
                                 AF.Gelu, bias=W(at, "ffn1_b", co=(oc, oc + 1)))
        f2 = sb([128, 2 * T], F32R, tag="t_f2")
        for oc in range(2):
            fp = ps_sm()
            for kc in range(4):
                mm(fp[:, 0:T],
                   W(wt, "ffn2", co=(128 * (kc * 2 + oc), 128 * (kc * 2 + oc + 1))),
                   f1[:, T * kc:T * (kc + 1)], start=(kc == 0), stop=(kc == 3))
            nc.scalar.activation(f2[:, T * oc:T * (oc + 1)], fp[:, 0:T],
                                 AF.Gelu, bias=W(at, "ffn2_b", co=(oc, oc + 1)))
        f3p = ps_sm()
        for kc in range(2):
            mm(f3p[:, 0:T], W(wt, "ffn3", co=(128 * kc, 128 * (kc + 1))),
               f2[:, T * kc:T * (kc + 1)], start=(kc == 0), stop=(kc == 1))
        f3 = sb([128, T], F32, tag="t_f3")
        nc.vector.tensor_scalar(f3[:], f3p[:, 0:T], W(at, "ffn3_b"), None,
                                op0=ALU.add)
        res3 = sb([128, T], F32R, tag="t_fres")
        nc.vector.tensor_tensor(res3[:], f3[:], t_ua[:].bitcast(F32),
                                op=ALU.add)
        tT = ln_fm(res3[:], T, W(at, "ln"), "t_ln")

    # ------------------------------------------------ pooling + output ----
    def pool_vec(xT, tok, tag, out_ap):
        a1p = ps_sm()
        mm(a1p[0:64, 0:tok], W("wp", "a1"), xT[:])
        a1 = sb([64, tok], F32R, tag=f"{tag}_a1")
        nc.scalar.activation(a1[:], a1p[0:64, 0:tok], AF.Tanh,
                             bias=W("ap_", "a1_b"))
        a2p = ps_s()
        mm(a2p[0:1, 0:tok], W("wp", "a2"), a1[:])
        arow = sb([1, tok], F32, tag=f"{tag}_ar")
        nc.vector.tensor_scalar(arow[:], a2p[0:1, 0:tok], W("ap_", "a2_b"),
                                None, op0=ALU.add)
        wrow = softmax_inplace(arow[:], 1, tok, f"{tag}_w")
        wbc = ps_c()
        mm(wbc[:, 0:tok], ones_row, wrow[:])
        pm = sb([128, tok], F32, tag=f"{tag}_pm")
        nc.vector.tensor_tensor(pm[:], xT[:].bitcast(F32), wbc[:, 0:tok],
                                op=ALU.mult)
        nc.vector.tensor_reduce(out_ap, pm[:], axis=AX.X, op=ALU.add)

    g2 = sb([128, 2], F32R, tag="g2")
    pool_vec(uT, U, "pl_u", g2[:, 0:1])
    pool_vec(tT, T, "pl_t", g2[:, 1:2])

    def swap2(x2, tag):
        o = sb([128, 2], F32R, tag=tag)
        nc.vector.tensor_copy(o[:, 0:1], x2[:, 1:2].bitcast(F32))
        nc.vector.tensor_copy(o[:, 1:2], x2[:, 0:1].bitcast(F32))
        return o

    def lin256(wname, bname, x2, func, tag):
        """y2 [128,2] = func(W @ x256 + b), x2 cols = chunks."""
        xsw = swap2(x2, f"{tag}_sw")
        y2 = sb([128, 2], F32R, tag=f"{tag}_y")
        for oc in range(2):
            pp = ps_sm()
            mm(pp[:, 0:2], W("wp", wname, co=(128 * oc, 128 * (oc + 1))),
               x2[:], start=True, stop=False)
            mm(pp[:, 0:2], W("wp", wname, co=(128 * (2 + oc), 128 * (3 + oc))),
               xsw[:], start=False, stop=True)
            if func == AF.Identity:
                nc.vector.tensor_scalar(y2[:, oc:oc + 1], pp[:, 0:1],
                                        W("ap_", bname, co=(oc, oc + 1)),
                                        None, op0=ALU.add)
            else:
                nc.scalar.activation(y2[:, oc:oc + 1], pp[:, 0:1], func,
                                     bias=W("ap_", bname, co=(oc, oc + 1)))
        return y2

    gy = lin256("pout", "pout_b", g2, AF.Identity, "g_p")
    # LN over 256 on [128,2]
    sp = ps_s()
    mm(sp[0:1, 0:2], ones_col, gy[:])
    sq = sb([128, 2], F32R, tag="g_sq")
    nc.vector.tensor_tensor(sq[:], gy[:], gy[:], op=ALU.mult)
    mm(sp[0:1, 2:4], ones_col, sq[:])
    sps = sb([1, 4], F32, tag="g_sp")
    nc.vector.tensor_copy(sps[:], sp[0:1, 0:4])
    st = sb([1, 6], F32, tag="g_st")
    nc.vector.tensor_tensor(st[0:1, 0:1], sps[0:1, 0:1], sps[0:1, 1:2],
                            op=ALU.add)   # sum
    nc.vector.tensor_tensor(st[0:1, 1:2], sps[0:1, 2:3], sps[0:1, 3:4],
                            op=ALU.add)   # sumsq
    nc.vector.tensor_scalar(st[0:1, 0:2], st[0:1, 0:2], 1.0 / 256, None,
                            op0=ALU.mult)  # mean, meansq
    nc.vector.tensor_tensor(st[0:1, 2:3], st[0:1, 0:1], st[0:1, 0:1],
                            op=ALU.mult)
    nc.vector.tensor_tensor(st[0:1, 2:3], st[0:1, 1:2], st[0:1, 2:3],
                            op=ALU.subtract)  # var
    nc.scalar.activation(st[0:1, 3:4], st[0:1, 2:3], AF.Sqrt, bias=1e-5)
    rs = sb([1, 2], F32R, tag="g_rs")
    nc.vector.tensor_copy(rs[0:1, 0:1], st[0:1, 0:1])     # mean
    nc.vector.reciprocal(rs[0:1, 1:2], st[0:1, 3:4])      # rstd
    mb = ps_c()
    mm(mb[:, 0:2], ones_row, rs[0:1, 0:2])  # col0 mean bcast, col1 rstd bcast
    xn = sb([128, 2], F32, tag="g_xn")
    nc.vector.tensor_tensor(xn[:], gy[:].bitcast(F32),
                            mb[:, 0:1].broadcast_to([128, 2]), op=ALU.subtract)
    nc.vector.tensor_tensor(xn[:], xn[:],
                            mb[:, 1:2].broadcast_to([128, 2]), op=ALU.mult)
    gn = sb([128, 2], F32R, tag="g_n")
    nc.vector.tensor_tensor(xn[:], xn[:], W("ap_", "pln", co=(0, 2)),
                            op=ALU.mult)
    nc.vector.tensor_tensor(gn[:], xn[:], W("ap_", "pln", co=(2, 4)),
                            op=ALU.add)
    e1 = lin256("el1", "el1_b", gn, AF.Gelu, "g_e1")
    e2 = lin256("el2", "el2_b", e1, AF.Identity, "g_e2")
    gfin = sb([128, 2], F32, tag="g_f")
    nc.vector.tensor_tensor(gfin[:], gn[:].bitcast(F32), e2[:].bitcast(F32),
                            op=ALU.add)

    u_tm = transpose(uT[:], 128, U, "out_u")
    t_tm = transpose(tT[:], 128, T, "out_t")
    nc.sync.dma_start(aps["u_out"][:], u_tm[:].bitcast(F32))
    nc.sync.dma_start(aps["t_out"][:], t_tm[:].bitcast(F32))
    nc.sync.dma_start(aps["g_out"].rearrange("(c p) o -> p (c o)", c=2),
                      gfin[:])


# ------------------------------------------------------------ build once ----
_PROG = None


def _build():
    global _PROG
    if _PROG is not None:
        return _PROG
    from contextlib import ExitStack
    nc = bacc.Bacc("TRN2", target_bir_lowering=False, debug=False,
                   num_devices=N_CORES)
    epst = nc.alloc_sbuf_tensor("const-eps", [128, 1], F32)
    nc.gpsimd.memset(epst.ap(), 1e-5)
    nc.const_aps.aps[(F32, 1e-5)] = epst.ap()
    nc.all_engine_barrier()
    aps = {}
    for cname, ch in CHUNKS.items():
        aps[cname] = nc.dram_tensor(cname, [128, ch.cols], ch.dt,
                                    kind="ExternalInput").ap()
    for name, (shape, dt) in BATCH_INPUTS.items():
        aps[name] = nc.dram_tensor(name, shape, dt, kind="ExternalInput").ap()
    for name, (shape, dt) in OUTPUTS.items():
        aps[name] = nc.dram_tensor(name, shape, dt, kind="ExternalOutput").ap()
    with tile.TileContext(nc) as tc:
        with ExitStack() as ctx:
            with nc.allow_low_precision("f32r rounding is intentional"):
                emit(nc, tc, ctx, aps)
    nc.compile()
    _PROG = nc
    return nc


# ------------------------------------------------------------ host prep ----
def _np(x):
    return np.asarray(x, dtype=np.float32)


def _pe_table_np():
    pos = np.arange(200, dtype=np.float32)[:, None]
    div = np.exp(np.arange(0, E, 2, dtype=np.float32) * (-math.log(10000.0) / E))
    pe = np.zeros((200, E), dtype=np.float32)
    pe[:, 0::2] = np.sin(pos * div)
    pe[:, 1::2] = np.cos(pos * div)
    return pe


def pack_inputs(usv_features, task_features, usv_task_edges, params):
    pet = _pe_table_np()
    shared = {}
    for cname, ch in CHUNKS.items():
        shared[cname] = np.zeros((128, ch.cols), np.float32)

    def put(c, e, arr):
        rows, off, cols = CHUNKS[c].entries[e]
        a = _np(arr)
        if a.ndim == 1:
            a = a[:, None]
        assert a.shape == (rows, cols), (c, e, a.shape, (rows, cols))
        shared[c][0:rows, off:off + cols] = a

    put("wc", "ident", np.eye(128, dtype=np.float32))
    put("wc", "ones_col", np.ones((128, 1), np.float32))
    _, off, _ = CHUNKS["wc"].entries["ones_row"]
    shared["wc"][0:1, off:off + 128] = 1.0
    ptab = np.zeros((128, 256), np.float32)
    ptab[0:100, 0:128] = pet[0:100]
    ptab[0:100, 128:256] = pet[100:200]
    put("wc", "petab", ptab)
    _, off, _ = CHUNKS["ac"].entries["iota"]
    shared["ac"][0:16, off:off + 200] = np.arange(200, dtype=np.float32)[None, :]

    for i in range(L):
        pu_, pt_ = params["usv"][i], params["task"][i]
        kin = 4 if i == 0 else 128
        w = _np(pu_["usv_proj"]["w"])
        wpad = np.zeros((kin, 128), np.float32)
        wpad[0:w.shape[1], :] = w.T
        put(f"wu{i}", "uproj", wpad)
        put(f"au{i}", "uproj_b", _np(pu_["usv_proj"]["b"]))
        for p, key in (("sa", "self_attn"), ("ca", "task_attn")):
            mha = pu_[key]
            for m in ("wq", "wk", "wv"):
                put(f"wu{i}", f"{p}_{m}", _np(mha[m]["w"]).T)
            put(f"wu{i}", f"{p}_wo", _np(mha["wo"]["w"]).T)
            put(f"au{i}", f"{p}_wo_b", _np(mha["wo"]["b"]))
            put(f"au{i}", f"{p}_ln",
                np.stack([_np(mha["ln_g"]), _np(mha["ln_b"])], 1))
        wtp = _np(pu_["task_proj"]["w"])
        tin_real = wtp.shape[1] - 3
        wt_t = np.zeros((kin, 128), np.float32)
        wt_t[0:tin_real, :] = wtp[:, 0:tin_real].T
        put(f"wu{i}", "wt_t", wt_t)
        wt_e = np.zeros((4, 128), np.float32)
        wt_e[0:3, :] = wtp[:, tin_real:].T
        put(f"wu{i}", "wt_e", wt_e)
        put(f"au{i}", "tp_b", _np(pu_["task_proj"]["b"]))
        put(f"wu{i}", "ffn1", _np(pu_["ffn1"]["w"]).T)
        put(f"au{i}", "ffn1_b", _np(pu_["ffn1"]["b"]).reshape(2, 128).T)
        put(f"wu{i}", "ffn2", _np(pu_["ffn2"]["w"]).T.reshape(2, 128, 128)
            .transpose(1, 0, 2).reshape(128, 256))
        put(f"au{i}", "ffn2_b", _np(pu_["ffn2"]["b"]))
        put(f"au{i}", "ln", np.stack([_np(pu_["ln_g"]), _np(pu_["ln_b"])], 1))

        w = _np(pt_["task_proj"]["w"])
        wpad = np.zeros((kin, 128), np.float32)
        wpad[0:w.shape[1], :] = w.T
        put(f"wt{i}", "tproj", wpad)
        put(f"at{i}", "tproj_b", _np(pt_["task_proj"]["b"]))
        mha = pt_["self_attn"]
        for m in ("wq", "wk", "wv"):
            put(f"wt{i}", f"sa_{m}", _np(mha[m]["w"]).T)
        put(f"wt{i}", "sa_wo", _np(mha["wo"]["w"]).T)
        put(f"at{i}", "sa_wo_b", _np(mha["wo"]["b"]))
        put(f"at{i}", "sa_ln",
            np.stack([_np(mha["ln_g"]), _np(mha["ln_b"])], 1))
        mha = pt_["usv_attn"]
        wu_ = _np(pt_["usv_proj"]["w"])
        bu_ = _np(pt_["usv_proj"]["b"])
        put(f"wt{i}", "ua_wq", _np(mha["wq"]["w"]).T)
        wk_, wv_ = _np(mha["wk"]["w"]), _np(mha["wv"]["w"])
        put(f"wt{i}", "ua_wku", (wk_ @ wu_).T)
        put(f"at{i}", "ua_kb", wk_ @ bu_)
        put(f"wt{i}", "ua_wvu", (wv_ @ wu_).T)
        put(f"at{i}", "ua_vb", wv_ @ bu_)
        put(f"wt{i}", "ua_wo", _np(mha["wo"]["w"]).T)
        put(f"at{i}", "ua_wo_b", _np(mha["wo"]["b"]))
        put(f"at{i}", "ua_ln",
            np.stack([_np(mha["ln_g"]), _np(mha["ln_b"])], 1))
        put(f"wt{i}", "ffn1", _np(pt_["ffn1"]["w"]).T)
        put(f"at{i}", "ffn1_b", _np(pt_["ffn1"]["b"]).reshape(4, 128).T)
        w2 = _np(pt_["ffn2"]["w"]).T
        put(f"wt{i}", "ffn2", w2.reshape(4, 128, 2, 128).transpose(1, 0, 2, 3)
            .reshape(128, 1024))
        put(f"at{i}", "ffn2_b", _np(pt_["ffn2"]["b"]).reshape(2, 128).T)
        put(f"wt{i}", "ffn3", _np(pt_["ffn3"]["w"]).T.reshape(2, 128, 128)
            .transpose(1, 0, 2).reshape(128, 256))
        put(f"at{i}", "ffn3_b", _np(pt_["ffn3"]["b"]))
        put(f"at{i}", "ln", np.stack([_np(pt_["ln_g"]), _np(pt_["ln_b"])], 1))

    pp = params["pool"]
    put("wp", "a1", _np(pp["a1"]["w"]).T)
    put("ap_", "a1_b", _np(pp["a1"]["b"]))
    put("wp", "a2", _np(pp["a2"]["w"]).T)
    put("ap_", "a2_b", _np(pp["a2"]["b"]).reshape(1, 1))
    put("wp", "pout", _np(pp["out"]["w"]).T.reshape(2, 128, 2, 128)
        .transpose(1, 0, 2, 3).reshape(128, 512))
    put("ap_", "pout_b", _np(pp["out"]["b"]).reshape(2, 128).T)
    put("ap_", "pln", np.concatenate(
        [_np(pp["ln_g"]).reshape(2, 128).T,
         _np(pp["ln_b"]).reshape(2, 128).T], 1))
    for nm, key in (("el1", "l1"), ("el2", "l2")):
        put("wp", nm, _np(params["enh"][key]["w"]).T.reshape(2, 128, 2, 128)
            .transpose(1, 0, 2, 3).reshape(128, 512))
        put("ap_", f"{nm}_b", _np(params["enh"][key]["b"]).reshape(2, 128).T)

    uf = _np(usv_features)
    tf = _np(task_features)
    ed = _np(usv_task_edges)
    per_batch = []
    for b in range(B):
        d = {}
        e = np.zeros((4, UT), np.float32)
        e[0:3, :] = ed[b].reshape(UT, 3).T
        d["edgesT"] = e
        x = np.zeros((4, U), np.float32)
        x[0:3, :] = uf[b].T
        d["u0T"] = x
        x = np.zeros((4, T), np.float32)
        x[0:4, :] = tf[b].T
        d["t0T"] = x
        idx = np.clip((uf[b, :, 0:2] * 100.0).astype(np.int32), 0, 199)
        d["pe_u0"] = (pet[idx[:, 0]] + pet[idx[:, 1]]).T.copy()
        tpos = tf[b, :, 0:2]
        idx = np.clip((tpos * 100.0).astype(np.int32), 0, 199)
        d["pe_task"] = (pet[idx[:, 0]] + pet[idx[:, 1]]).T.copy()
        d2 = ((tpos[:, None, :] - tpos[None, :, :]) ** 2).sum(-1)
        dist = np.sqrt(d2, dtype=np.float32)
        nidx = np.argsort(dist, axis=-1, kind="stable")[:, :5]
        bias = np.full((T, T), -1e30, np.float32)
        np.put_along_axis(bias, nidx, 0.0, axis=-1)
        d["knn"] = np.tile(bias, (2, 1))
        per_batch.append(d)
    return shared, per_batch


# --------------------------------------------------------------- kernel ----
def kernel(usv_features, task_features, usv_task_edges, params):
    nc = _build()
    shared, per_batch = pack_inputs(usv_features, task_features,
                                    usv_task_edges, params)
    in_maps = [{**shared, **per_batch[c % B]} for c in range(N_CORES)]
    res = run_bass_kernel_spmd(nc, in_maps, core_ids=list(range(N_CORES)))
    u = np.stack([res.results[b]["u_out"] for b in range(B)])
    t = np.stack([res.results[b]["t_out"] for b in range(B)])
    g = np.stack([res.results[b]["g_out"][:, 0] for b in range(B)])
    return u, t, g
